# revision 30
# baseline (speedup 1.0000x reference)
"""Trainium2 Bass kernel for the GAT-with-gated-residual block.

Computation (per batch b):
  h   = x @ W_w^T + W_b                       [N, D]
  e   = (h @ A) @ h^T;  e_sym = e + e^T       [N, N]
  att = softmax_axis1(where(adj>0, e_sym, -inf)) * adj
  hp  = relu(att @ h)                         [N, D]
  c   = sigmoid([x, hp] @ gate_w^T + gate_b)  [N, 1]
  out = c * x + (1 - c) * hp

Sharding: data-parallel over batch (4 batches per core, 8 cores).

Kernel strategy (per core, per batch), all in "transposed" orientation so the
softmax axis lands on the free dim:
  - xT via PE transpose; hT = W_wT-matmul; hAT = A-matmul (both [D, N]).
  - e_sym row-blocks [128, N] via two accumulating matmuls (e + e^T).
  - adj is cast-loaded f32->bf16 (SWDGE) and transposed on-chip via the
    DMA xbar (128x128 bf16 tiles) to get adjT (mask with m on partitions).
  - One DVE tensor_tensor_reduce computes tneg = -(e*adjT) and the
    per-partition running min = -max(0, colmax) in a single pass.
  - ACT exp with scale=-1, per-partition bias=negM and fused accum_out
    gives texp = exp(e*adjT - M) in bf16 plus the row sums s.
  - Softmax normalization is folded into h: hs = h * (1/s) per row (bf16).
  - h_prime = texp^T-matmul @ hs accumulated over 8 j-blocks, relu on evict.
  - Gate: x-part on PE (xT @ gwx), hp-part on GPSIMD (fused mul+reduce),
    tanh on ACT (sigmoid(z) = 0.5 + 0.5*tanh(z/2), stays in the exp table set).
  - Blend on GPSIMD: out = hp + coeff*(x - hp).
"""

import os
import numpy as np
from contextlib import ExitStack

import concourse.bass as bass
import concourse.bacc as bacc
import concourse.mybir as mybir
import concourse.tile as tile
from concourse.masks import make_identity

F32 = mybir.dt.float32
BF16 = mybir.dt.bfloat16
AF = mybir.ActivationFunctionType
OP = mybir.AluOpType

B, N, D = 32, 1024, 128
_STAGE = int(os.environ.get("KERNEL_STAGE", "99"))
NCORES = 8
BPC = B // NCORES          # batches per core
NB = N // 128              # 8 row/col blocks


def build_nc(reps=1):
    nc = bacc.Bacc()
    x_d = nc.dram_tensor("x", (BPC, N, D), F32, kind="ExternalInput")
    adj_d = nc.dram_tensor("adj", (BPC, N, N), F32, kind="ExternalInput")
    Ww_d = nc.dram_tensor("W_w", (D, D), F32, kind="ExternalInput")
    Wb_d = nc.dram_tensor("W_b", (D,), F32, kind="ExternalInput")
    A_d = nc.dram_tensor("A", (D, D), F32, kind="ExternalInput")
    gw_d = nc.dram_tensor("gate_w", (1, 2 * D), F32, kind="ExternalInput")
    gb_d = nc.dram_tensor("gate_b", (1,), F32, kind="ExternalInput")
    out_d = nc.dram_tensor("out", (BPC, N, D), F32, kind="ExternalOutput")

    with tile.TileContext(nc) as tc:
        with ExitStack() as ctx:
            _body(ctx, tc, nc, x_d, adj_d, Ww_d, Wb_d, A_d, gw_d, gb_d, out_d,
                  reps=reps)
    nc.finalize()
    return nc


def _body(ctx, tc, nc, x_d, adj_d, Ww_d, Wb_d, A_d, gw_d, gb_d, out_d, reps=1):
    const = ctx.enter_context(tc.tile_pool(name="const", bufs=1))
    adjn_pool = ctx.enter_context(tc.tile_pool(name="adjn", bufs=16))
    adjt_pool = ctx.enter_context(tc.tile_pool(name="adjt", bufs=16))
    texp_pool = ctx.enter_context(tc.tile_pool(name="texp", bufs=3))
    att_pool = ctx.enter_context(tc.tile_pool(name="att", bufs=16))
    big_pool = ctx.enter_context(tc.tile_pool(name="big", bufs=2))
    xn_pool = ctx.enter_context(tc.tile_pool(name="xn", bufs=16))
    sm_pool = ctx.enter_context(tc.tile_pool(name="sm", bufs=16))
    st_pool = ctx.enter_context(tc.tile_pool(name="st", bufs=4))
    out_pool = ctx.enter_context(tc.tile_pool(name="outp", bufs=8))
    ps_big = ctx.enter_context(tc.tile_pool(name="ps_big", bufs=1, space="PSUM"))
    ps_e = ctx.enter_context(tc.tile_pool(name="ps_e", bufs=2, space="PSUM"))
    ps_sm = ctx.enter_context(tc.tile_pool(name="ps_sm", bufs=2, space="PSUM"))

    # ---- constants -------------------------------------------------------
    ident = const.tile([128, 128], F32)
    make_identity(nc, ident)

    Ww_nat = const.tile([128, 128], F32)          # W_w[o, d], o on partitions
    nc.sync.dma_start(out=Ww_nat, in_=Ww_d[:, :])
    A_nat = const.tile([128, 128], F32)           # A[k, l], lhsT for hAT
    nc.sync.dma_start(out=A_nat, in_=A_d[:, :])

    # W_w^T via PE transpose (lhsT for hT matmul, rhs for h-nat matmul)
    ps0 = ps_sm.tile([128, 128], F32, tag="small")
    nc.tensor.transpose(ps0, Ww_nat, ident)
    WwT = const.tile([128, 128], F32)
    nc.vector.tensor_copy(WwT, ps0)

    # W_b as per-partition column [128, 1] (bias for hT via ACT bias)
    Wb_col = const.tile([128, 1], F32)
    nc.sync.dma_start(out=Wb_col, in_=Wb_d.rearrange("(p o) -> p o", o=1))
    # W_b broadcast [128, N]: repeated along free for the h-nat mega eviction
    Wb_bc = const.tile([128, N], F32)
    wb_ap = Wb_d.ap()
    wb_src = bass.AP(
        tensor=wb_ap.tensor, offset=wb_ap.offset,
        ap=[[0, 128], [0, NB], [1, D]],
    )
    nc.gpsimd.dma_start(out=Wb_bc.rearrange("p (b d) -> p b d", b=NB), in_=wb_src)

    # gate weights
    gwx_col = const.tile([128, 1], F32)
    nc.sync.dma_start(out=gwx_col, in_=gw_d[0, 0:D].rearrange("(p o) -> p o", o=1))
    gwh_bc = const.tile([128, 128], F32)
    g1 = gw_d[0:1, D:2 * D]
    gwh_src = bass.AP(tensor=g1.tensor, offset=g1.offset, ap=[[0, 128], g1.ap[-1]])
    nc.gpsimd.dma_start(out=gwh_bc, in_=gwh_src)
    gb_raw = const.tile([128, 1], F32)
    gb1 = gb_d[0:1]
    gb_src = bass.AP(tensor=gb1.tensor, offset=gb1.offset, ap=[[0, 128], [1, 1]])
    nc.gpsimd.dma_start(out=gb_raw, in_=gb_src)
    gb_half = const.tile([128, 1], F32)
    nc.vector.tensor_scalar_mul(gb_half, gb_raw, 0.5)
    shift_neg = const.tile([128, 1], F32)
    nc.vector.memset(shift_neg, -100.0)

    # ---- per-batch pipeline ---------------------------------------------
    for b in [bb for _ in range(reps) for bb in range(BPC)]:
        # adj cast-load (f32 -> bf16) and on-chip xbar transpose
        adj_nat = []
        for nb in range(NB):
            an = adjn_pool.tile([128, N], BF16, tag="adj_nat")
            nc.gpsimd.dma_start(out=an, in_=adj_d[b, nb * 128:(nb + 1) * 128, :])
            adj_nat.append(an)
        adjT = []
        for mb in range(NB):
            at = adjt_pool.tile([128, N], BF16, tag="adjT")
            adjT.append(at)
        _no_xbar = bool(os.environ.get("KERNEL_NO_XBAR"))
        for mb in range(NB if _STAGE >= 2 else 0):
            for nb in range(NB):
                nc.sync.dma_start(
                    out=adjT[mb][:, nb * 128:(nb + 1) * 128],
                    in_=adj_nat[nb][:, mb * 128:(mb + 1) * 128],
                    transpose=not _no_xbar,
                )

        # x loads + PE transpose -> xT [D, N]
        xn = []
        for ib in range(NB):
            xt = xn_pool.tile([128, D], F32, tag="xn")
            nc.sync.dma_start(out=xt, in_=x_d[b, ib * 128:(ib + 1) * 128, :])
            xn.append(xt)
        if _STAGE < 3:
            for ib in range(NB):
                ob0 = out_pool.tile([128, D], F32, tag="ob")
                nc.vector.tensor_copy(ob0, xn[ib])
                nc.sync.dma_start(out=out_d[b, ib * 128:(ib + 1) * 128, :], in_=ob0)
            continue
        ps_xT = ps_big.tile([128, N], F32, tag="mega")
        for ib in range(NB):
            nc.tensor.transpose(ps_xT[:, ib * 128:(ib + 1) * 128], xn[ib], ident)
        xT = big_pool.tile([128, N], F32, tag="xT")
        nc.scalar.copy(out=xT, in_=ps_xT)

        # hT = W_w @ x^T + W_b  [o, n]
        ps_hT = ps_big.tile([128, N], F32, tag="mega")
        for half in range(2):
            sl = slice(half * 512, half * 512 + 512)
            nc.tensor.matmul(ps_hT[:, sl], WwT, xT[:, sl], start=True, stop=True)
        hT = big_pool.tile([128, N], F32, tag="hT")
        nc.scalar.activation(hT, ps_hT, AF.Identity, bias=Wb_col, scale=1.0)

        # hAT = A^T-contract  [l, n]
        ps_hAT = ps_big.tile([128, N], F32, tag="mega")
        for half in range(2):
            sl = slice(half * 512, half * 512 + 512)
            nc.tensor.matmul(ps_hAT[:, sl], A_nat, hT[:, sl], start=True, stop=True)
        hAT = big_pool.tile([128, N], F32, tag="hAT")
        nc.scalar.copy(out=hAT, in_=ps_hAT)

        # h natural blocks (mega layout [p, (ib, d)]) + bias on eviction
        ps_hn = ps_big.tile([128, N], F32, tag="mega")
        for ib in range(NB):
            sl = slice(ib * 128, ib * 128 + 128)
            nc.tensor.matmul(ps_hn[:, sl], xT[:, sl], WwT, start=True, stop=True)
        hn = big_pool.tile([128, N], F32, tag="hn")
        nc.vector.tensor_tensor(out=hn, in0=ps_hn, in1=Wb_bc, op=OP.add)

        if _STAGE < 4:
            continue
        # e_sym blocks; texp = exp(e - 100) straight from PSUM (ACT);
        # att = texp * adjT with fused row-sum accum (DVE, all-bf16)
        s_all = st_pool.tile([128, NB], F32, tag="s_all")
        att = []
        for mb in range(NB):
            msl = slice(mb * 128, mb * 128 + 128)
            pse = ps_e.tile([128, N], F32, tag="e")
            for half in range(2):
                sl = slice(half * 512, half * 512 + 512)
                nc.tensor.matmul(pse[:, sl], hAT[:, msl], hT[:, sl],
                                 start=True, stop=False)
                nc.tensor.matmul(pse[:, sl], hT[:, msl], hAT[:, sl],
                                 start=False, stop=True)
            tx = texp_pool.tile([128, N], BF16, tag="texp")
            nc.scalar.activation(tx, pse, AF.Exp, bias=shift_neg, scale=1.0)
            av = att_pool.tile([128, N], BF16, tag="att")
            nc.vector.scalar_tensor_tensor(
                out=av, in0=tx, scalar=1.0, in1=adjT[mb],
                op0=OP.mult, op1=OP.mult, accum_out=s_all[:, mb:mb + 1],
            )
            att.append(av)

        if _STAGE < 5:
            continue
        # softmax scale folded into h: hs = h * (1/s)
        recip = st_pool.tile([128, NB], F32, tag="recip")
        nc.vector.reciprocal(recip, s_all)
        hs = []
        for ib in range(NB):
            hv = sm_pool.tile([128, D], BF16, tag="hs")
            nc.vector.tensor_scalar_mul(
                hv, hn[:, ib * 128:(ib + 1) * 128], recip[:, ib:ib + 1])
            hs.append(hv)

        # gate x-part on PE (before hp loop so the psum slot frees early)
        ps_g = ps_sm.tile([128, NB], F32, tag="small")
        for ib in range(NB):
            nc.tensor.matmul(ps_g[:, ib:ib + 1], xT[:, ib * 128:(ib + 1) * 128],
                             gwx_col, start=True, stop=True)
        gx = st_pool.tile([128, NB], F32, tag="gx")
        nc.vector.tensor_copy(gx, ps_g)

        # h_prime = relu(att @ h) ; gate-h fused reduce; blend; store
        hp = []
        gh = st_pool.tile([128, NB], F32, tag="gh")
        for ib in range(NB):
            isl = slice(ib * 128, ib * 128 + 128)
            psh = ps_sm.tile([128, 128], F32, tag="small")
            for jb in range(NB):
                nc.tensor.matmul(psh, att[jb][:, isl], hs[jb],
                                 start=(jb == 0), stop=(jb == NB - 1))
            hv = sm_pool.tile([128, D], F32, tag="hp")
            if ib % 2 == 0:
                nc.scalar.activation(hv, psh, AF.Relu)
            else:
                nc.vector.tensor_scalar_max(hv, psh, 0.0)
            hp.append(hv)
            scr = sm_pool.tile([128, D], F32, tag="gscr")
            nc.vector.scalar_tensor_tensor(
                out=scr, in0=hv, scalar=1.0, in1=gwh_bc,
                op0=OP.mult, op1=OP.mult, accum_out=gh[:, ib:ib + 1])
        glin = st_pool.tile([128, NB], F32, tag="glin")
        nc.vector.tensor_tensor(out=glin, in0=gx, in1=gh, op=OP.add)
        tau = st_pool.tile([128, NB], F32, tag="tau")
        nc.scalar.activation(tau, glin, AF.Tanh, bias=gb_half, scale=0.5)
        coeff = st_pool.tile([128, NB], F32, tag="coeff")
        nc.vector.tensor_scalar(out=coeff, in0=tau, scalar1=0.5, scalar2=0.5,
                                op0=OP.mult, op1=OP.add)

        if _STAGE < 6:
            continue
        for ib in range(NB):
            dd = sm_pool.tile([128, D], F32, tag="dd")
            nc.gpsimd.tensor_sub(dd, xn[ib], hp[ib])
            ob = out_pool.tile([128, D], F32, tag="ob")
            nc.vector.scalar_tensor_tensor(
                out=ob, in0=dd, scalar=coeff[:, ib:ib + 1], in1=hp[ib],
                op0=OP.mult, op1=OP.add)
            nc.sync.dma_start(out=out_d[b, ib * 128:(ib + 1) * 128, :], in_=ob)


def kernel(**inputs):
    from concourse.bass_utils import run_bass_kernel_spmd

    nc = build_nc()
    x = np.ascontiguousarray(inputs["x"], dtype=np.float32)
    adj = np.ascontiguousarray(inputs["adj"], dtype=np.float32)
    shared = {
        "W_w": np.ascontiguousarray(inputs["W_w"], dtype=np.float32),
        "W_b": np.ascontiguousarray(inputs["W_b"], dtype=np.float32),
        "A": np.ascontiguousarray(inputs["A"], dtype=np.float32),
        "gate_w": np.ascontiguousarray(inputs["gate_w"], dtype=np.float32),
        "gate_b": np.ascontiguousarray(inputs["gate_b"], dtype=np.float32),
    }
    in_maps = []
    for c in range(NCORES):
        sl = slice(c * BPC, (c + 1) * BPC)
        in_maps.append({"x": x[sl], "adj": adj[sl], **shared})
    res = run_bass_kernel_spmd(nc, in_maps, core_ids=list(range(NCORES)))
    return np.concatenate([r["out"] for r in res.results], axis=0)
